# revision 4
# baseline (speedup 1.0000x reference)
"""Trainium2 Bass kernel for nn_CausalAttention_73212012527759.

Math per position p (8192 of them): q,k,v blocks [H=16, E=64]:
    Q = T1 @ vec(q)   (fused E-rfft 9-bin + H-DFT)      [288 = 3area*96]
    M1 = Q*K, M2 = Q*K_ri_swapped (complex-product parts)  fp16
    corr = T2 @ [M1;M2]  (inverse H-DFT + cropped irfft16 + 1/sqrt(E))
           rows r = k*16 + a   [256 over 2 psum halves]
    E'' = exp(corr + logW[r]);  S[a] = sum_k E''[k,a]/W[a,k];  R = 1/S
    rrep = SEL @ R    (partition-broadcast of R[a] to all 256 r-rows)
    corrF = E'' * rrep  (fp16)
    out[a, d] = sum_k corrF[k,a] * v[k, d]

v2 changes vs baseline:
  - q,k,v host-cast to fp16; ONE whole-rep DMA each (3 vs ~24 loads).
  - rrep DRAM bounce (9 DMAs/tile) replaced by one selector matmul.
  - step5: 8 positions packed block-diagonally into one 128x128 fp16
    stationary (pre-zeroed SBUF + strided-AP cols) -> 32 ldw + 32 mm
    per 256-pos tile instead of 256+256 tiny ones.
  - out in fp16, exact packing (2MB vs 8MB), 1 DMA/tile.

Position mapping: core c, tile t, diag-slot i (8), group g (32):
    global_pos = c*1024 + t*256 + i*32 + g
"""

import math
import numpy as np

import concourse.bass as bass
import concourse.bacc as bacc
import concourse.mybir as mybir
from concourse import tile
from concourse.bass_utils import run_bass_kernel_spmd

B, L, H, E = 4, 2048, 16, 64
NB = 9
NPOS = B * L
NCORES = 8
P_CORE = NPOS // NCORES      # 1024
TILE_P = 256
NT = P_CORE // TILE_P        # 4
NG = 32                      # groups of 8 positions per tile
ORDER = 0.2
SCALE = 1.0 / math.sqrt(E)
FP32 = mybir.dt.float32
FP32R = mybir.dt.float32r
FP16 = mybir.dt.float16

REPS = 1  # repeat compute in-NEFF (for timing)


def _build_constants():
    e = np.arange(E)[:, None]
    f = np.arange(NB)[None, :]
    Cc = np.exp(-2j * np.pi * e * f / E)
    g = np.arange(H)[:, None]
    h = np.arange(H)[None, :]
    Fc = np.exp(-2j * np.pi * g * h / H)
    T1 = np.zeros((H * E, 288))
    prod = np.einsum('gh,ef->hegf', Fc, Cc)
    for ff in range(NB):
        for ri in range(2):
            for gg in range(H):
                col = ff * 32 + ri * 16 + gg
                vals = prod[:, :, gg, ff]
                T1[:, col] = (vals.real if ri == 0 else vals.imag).reshape(-1)

    R18 = np.zeros((2 * NB, 16))
    for ff in range(NB):
        b_ = np.zeros(NB, complex); b_[ff] = 1.0
        R18[ff] = np.fft.irfft(b_, n=16)
        b_ = np.zeros(NB, complex); b_[ff] = 1j
        R18[NB + ff] = np.fft.irfft(b_, n=16)

    a_ = np.arange(H)[:, None]
    IRe = np.cos(2 * np.pi * a_ * np.arange(H)[None, :] / H) / H
    IIm = np.sin(2 * np.pi * a_ * np.arange(H)[None, :] / H) / H
    T2 = np.zeros((576, 256))
    for ff in range(NB):
        for gg in range(H):
            cre = np.outer(IRe[:, gg], R18[ff]) + np.outer(IIm[:, gg], R18[NB + ff])
            cim = -np.outer(IIm[:, gg], R18[ff]) + np.outer(IRe[:, gg], R18[NB + ff])
            flat_re = (SCALE * cre).T.reshape(-1)   # index k*16+a
            flat_im = (SCALE * cim).T.reshape(-1)
            T2[ff * 32 + 0 * 16 + gg] += flat_re
            T2[ff * 32 + 1 * 16 + gg] += flat_re
            T2[288 + ff * 32 + 1 * 16 + gg] += flat_im
            T2[288 + ff * 32 + 0 * 16 + gg] -= flat_im

    jj = np.arange(1, H * H, dtype=np.float64)
    w = np.concatenate([[1.0], np.cumprod(np.abs(1.0 - (ORDER + 1.0) / jj))])
    W = w.reshape(H, H)
    logW = np.log(W)
    logW_rows = logW.T.reshape(-1)          # [256] at r=k*16+a
    O1 = np.zeros((256, 16))
    for k in range(H):
        for a in range(H):
            O1[k * 16 + a, a] = 1.0 / W[a, k]
    SEL = np.zeros((16, 128))
    for r in range(128):
        SEL[r % 16, r] = 1.0
    return (T1, T2, logW_rows.astype(np.float32), O1.astype(np.float32),
            SEL.astype(np.float32))


_CONSTS = None
def get_constants():
    global _CONSTS
    if _CONSTS is None:
        _CONSTS = _build_constants()
    return _CONSTS


SHUF_SWAP16 = list(range(16, 32)) + list(range(16))


def build_nc(reps=1):
    nc = bacc.Bacc("TRN2", target_bir_lowering=False, debug=False,
                   num_devices=NCORES)

    # q and k interleaved per (tile, chunk): [q 256 | k 256]
    qkT = nc.declare_dram_parameter("qkT", [128, NT * 8 * 2 * TILE_P], FP16,
                                    isOutput=False)
    vp = nc.declare_dram_parameter("vp", [128, NT * NG * E], FP16,
                                   isOutput=False)
    t1 = nc.declare_dram_parameter("t1", [128, 8 * 3 * 96], FP16,
                                   isOutput=False)
    t2 = nc.declare_dram_parameter("t2", [128, 6 * 2 * 128], FP16,
                                   isOutput=False)
    lw = nc.declare_dram_parameter("lw", [128, 2], FP32, isOutput=False)
    o1 = nc.declare_dram_parameter("o1", [128, 2 * 16], FP32R, isOutput=False)
    sel = nc.declare_dram_parameter("sel", [16, 128], FP32R, isOutput=False)
    # out rows = i'*16 + a, cols = (t, g, d)
    out = nc.declare_dram_parameter("out", [128, NT * NG * E], FP16,
                                    isOutput=True)

    with tile.TileContext(nc) as tc:
        with (
            tc.tile_pool(name="const", bufs=1) as cpool,
            tc.tile_pool(name="io", bufs=2) as io,
            tc.tile_pool(name="mid", bufs=2) as mid,
            tc.tile_pool(name="xcp", bufs=1) as xcp,
            tc.tile_pool(name="obp", bufs=2) as obp,
            tc.tile_pool(name="dram", bufs=2, space="DRAM") as dpool,
            tc.tile_pool(name="ps_qk", bufs=1, space="PSUM") as ps_qk,
            tc.tile_pool(name="ps_c", bufs=1, space="PSUM") as ps_c,
            tc.tile_pool(name="ps_s", bufs=1, space="PSUM") as ps_s,
            tc.tile_pool(name="ps_o", bufs=2, space="PSUM") as ps_o,
        ):
            t1_sb = cpool.tile([128, 8 * 3 * 96], FP16)
            nc.sync.dma_start(t1_sb[:], t1.ap())
            t2_sb = cpool.tile([128, 6 * 2 * 128], FP16)
            nc.sync.dma_start(t2_sb[:], t2.ap())
            lw_sb = cpool.tile([128, 2], FP32)
            nc.sync.dma_start(lw_sb[:], lw.ap())
            o1_sb = cpool.tile([128, 2 * 16], FP32R)
            nc.sync.dma_start(o1_sb[:], o1.ap())
            sel_sb = cpool.tile([16, 128], FP32R)
            nc.sync.dma_start(sel_sb[:], sel.ap())

            t1_4 = t1_sb[:].rearrange("p (c a m) -> p c a m", c=8, a=3)
            t2_4 = t2_sb[:].rearrange("p (m h x) -> p m h x", m=6, h=2)
            o1_3 = o1_sb[:].rearrange("p (h x) -> p h x", h=2)

            # two persistent xc buffers, zeroed once; the per-tile scatter
            # DMAs only ever write the 8 diagonal blocks, so the
            # off-diagonal zeros survive all later iterations.
            xc_bufs = []
            for bi in range(2):
                xct = xcp.tile([128, 8 * 16 * NG], FP16, tag=f"xc{bi}")
                nc.vector.memset(xct[:], 0.0)
                xc_bufs.append(xct)

            for rep in range(reps):
                qk_sb = io.tile([128, NT * 8 * 2 * TILE_P], FP16, tag="qk")
                xv = io.tile([128, NT * NG * E], FP16, tag="xv")
                nc.sync.dma_start(qk_sb[:], qkT.ap())
                nc.sync.dma_start(xv[:], vp.ap())
                qk_v = qk_sb[:].rearrange("p (t c h x) -> p t c h x",
                                          t=NT, c=8, h=2)
                xv_v = xv[:].rearrange("p (t g d) -> p t g d", t=NT, g=NG)

                for t in range(NT):
                    # ---- S24: Q/K = T1 @ q/k ----
                    qps = ps_qk.tile([128, 3 * TILE_P], FP32, tag="qps")
                    kps = ps_qk.tile([128, 3 * TILE_P], FP32, tag="kps")
                    for (qk_i, dst) in ((0, qps), (1, kps)):
                        for area in range(3):
                            for c in range(8):
                                nc.tensor.matmul(
                                    dst[0:96, bass.ts(area, TILE_P)],
                                    t1_4[:, c, area, :],
                                    qk_v[:, t, c, qk_i, :],
                                    start=(c == 0), stop=(c == 7))

                    # ---- products (fp16) ----
                    ksb = mid.tile([128, 3 * TILE_P], FP32, tag="ksb")
                    k2 = mid.tile([128, 3 * TILE_P], FP32, tag="k2")
                    m1 = mid.tile([128, 3 * TILE_P], FP16, tag="m1")
                    m2 = mid.tile([128, 3 * TILE_P], FP16, tag="m2")
                    nc.scalar.copy(ksb[0:96, :], kps[0:96, :])
                    nc.vector.tensor_mul(m1[0:96, :], qps[0:96, :],
                                         ksb[0:96, :])
                    nc.vector.stream_shuffle(k2[0:96, :], ksb[0:96, :],
                                             SHUF_SWAP16)
                    nc.vector.tensor_mul(m2[0:96, :], qps[0:96, :],
                                         k2[0:96, :])

                    # ---- S6: corr = T2 @ [M1;M2] ----
                    cps = ps_c.tile([128, 2 * TILE_P], FP32, tag="cps")
                    for half in range(2):
                        mi = 0
                        for (msrc, base) in ((m1, 0), (m2, 3)):
                            for area in range(3):
                                nc.tensor.matmul(
                                    cps[:, bass.ts(half, TILE_P)],
                                    t2_4[0:96, base + area, half, :],
                                    msrc[0:96, bass.ts(area, TILE_P)],
                                    start=(mi == 0), stop=(mi == 5))
                                mi += 1

                    # ---- softmax pieces ----
                    esb = mid.tile([128, 2 * TILE_P], FP32R, tag="esb")
                    for half in range(2):
                        nc.scalar.activation(
                            esb[:, bass.ts(half, TILE_P)],
                            cps[:, bass.ts(half, TILE_P)],
                            mybir.ActivationFunctionType.Exp,
                            bias=lw_sb[:, half:half + 1], scale=1.0)
                    srp = ps_s.tile([128, 2 * TILE_P], FP32, tag="srp")
                    sps = srp[:, 0:TILE_P]
                    rrp = srp[:, TILE_P:2 * TILE_P]
                    for half in range(2):
                        nc.tensor.matmul(
                            sps[0:16, :], o1_3[:, half, :],
                            esb[:, bass.ts(half, TILE_P)],
                            start=(half == 0), stop=(half == 1))
                    rsb = mid.tile([128, TILE_P], FP32R, tag="rsb")
                    with nc.allow_low_precision(
                            reason="1/S feeds an fp32r matmul broadcast"):
                        nc.vector.reciprocal(rsb[0:16, :], sps[0:16, :])

                    # ---- rrep[r, pos] = R[r%16, pos] via selector matmul ----
                    nc.tensor.matmul(rrp, sel_sb[:], rsb[0:16, :],
                                     start=True, stop=True)

                    # ---- corrF (fp16) = E'' * rrep ----
                    cf = mid.tile([128, 2 * TILE_P], FP16, tag="cf")
                    for half in range(2):
                        nc.vector.tensor_mul(
                            cf[:, bass.ts(half, TILE_P)],
                            esb[:, bass.ts(half, TILE_P)].bitcast(FP32),
                            rrp)

                    # ---- bounce: cf -> DRAM [r=k*16+a, pos] ----
                    cf_d = dpool.tile([2 * 128, TILE_P], FP16, tag="cfd")
                    nc.sync.dma_start(
                        cf_d[:].rearrange("(h r) x -> r h x", h=2),
                        cf[:].rearrange("r (h x) -> r h x", h=2))

                    # ---- xc readback: block-diagonal scatter, 8 DMAs ----
                    # xc[16i+k, i*512 + a*32 + g] = cf_d[k*16+a, i*32+g]
                    xc = xc_bufs[(rep * NT + t) % 2]
                    cfd_v = cf_d[:].rearrange("(k a) (i g) -> k a i g",
                                              a=16, g=NG)
                    for i in range(8):
                        eng = nc.scalar if (i % 2) else nc.sync
                        eng.dma_start(
                            xc[16 * i:16 * i + 16,
                               bass.ts(i, 16 * NG)].rearrange(
                                   "k (a g) -> k a g", g=NG),
                            cfd_v[:, :, i, :])

                    # ---- step5: out = corrF^T @ v, 8 positions per matmul ----
                    xc_v = xc[:].rearrange("p (i a g) -> p i a g", i=8, a=16)
                    ob = obp.tile([128, NG * E], FP16, tag="ob")
                    for gb in range(4):
                        ops = ps_o.tile([128, 8 * E], FP32, tag="ops")
                        for gg in range(8):
                            g = gb * 8 + gg
                            nc.tensor.matmul(
                                ops[:, bass.ts(gg, E)],
                                xc_v[:, :, :, g],
                                xv_v[:, t, g, :],
                                start=True, stop=True)
                        if gb % 2:
                            nc.vector.tensor_scalar_mul(
                                ob[:, bass.ts(gb, 8 * E)], ops[:], 1.0)
                        else:
                            nc.scalar.copy(ob[:, bass.ts(gb, 8 * E)], ops[:])
                    nc.sync.dma_start(
                        out.ap()[:, bass.ts(t, NG * E)], ob[:])

    nc.compile()
    return nc


_NC = {}
def get_nc(reps=1):
    if reps not in _NC:
        _NC[reps] = build_nc(reps)
    return _NC[reps]


def make_in_maps(q, k, v):
    """q,k,v: [NPOS, H, E] fp32 -> list of per-core input dicts."""
    T1c, T2c, logWc, O1c, SELc = get_constants()
    t1_img = np.ascontiguousarray(
        T1c.reshape(8, 128, 288).transpose(1, 0, 2).reshape(128, 8, 3, 96)
        .reshape(128, -1)).astype(np.float16)
    t2_img = np.ascontiguousarray(
        T2c.reshape(6, 96, 2, 128).transpose(1, 0, 2, 3).reshape(96, -1))
    t2_img = np.concatenate(
        [t2_img, np.zeros((32, t2_img.shape[1]), t2_img.dtype)],
        axis=0).astype(np.float16)
    lw_img = np.ascontiguousarray(logWc.reshape(2, 128).T)
    o1_img = np.ascontiguousarray(
        O1c.reshape(2, 128, 16).transpose(1, 0, 2).reshape(128, 32))
    sel_img = SELc

    q16 = q.astype(np.float16).reshape(NCORES, NT, TILE_P, H * E)
    k16 = k.astype(np.float16).reshape(NCORES, NT, TILE_P, H * E)
    v16 = v.astype(np.float16).reshape(NCORES, NT, 8, NG, H, E)

    in_maps = []
    for c in range(NCORES):
        # qkT[p, (t, chunk, {q,k}, pos)] = q/k[t, pos, row=chunk*128+p]
        qc = (q16[c].transpose(2, 0, 1)            # [row, t, pos]
              .reshape(8, 128, NT, TILE_P)         # [chunk, p, t, pos]
              .transpose(1, 2, 0, 3))              # [p, t, chunk, pos]
        kc = (k16[c].transpose(2, 0, 1)
              .reshape(8, 128, NT, TILE_P)
              .transpose(1, 2, 0, 3))
        qkc = np.ascontiguousarray(
            np.stack([qc, kc], axis=3)             # [p, t, chunk, 2, pos]
            .reshape(128, -1))
        # vp[16i+k, (t, g, d)] = v[t, i, g, k, d]
        vc = np.ascontiguousarray(
            v16[c].transpose(1, 3, 0, 2, 4)        # [i, k, t, g, d]
            .reshape(128, -1))
        in_maps.append({
            "qkT": qkc, "vp": vc,
            "t1": t1_img, "t2": t2_img, "lw": lw_img, "o1": o1_img,
            "sel": sel_img,
        })
    return in_maps


def unpack_out(results):
    outs = []
    for c in range(NCORES):
        o = results[c]["out"].astype(np.float32)
        o = o.reshape(8, 16, NT, NG, E)            # [i, a, t, g, d]
        oc = o.transpose(2, 0, 3, 1, 4).reshape(P_CORE, H, E)
        outs.append(oc)
    return np.concatenate(outs, axis=0).reshape(B, L, H, E)


def kernel(queries, keys, values, attn_mask=None):
    q = np.ascontiguousarray(queries, dtype=np.float32).reshape(NPOS, H, E)
    k = np.ascontiguousarray(keys, dtype=np.float32).reshape(NPOS, H, E)
    v = np.ascontiguousarray(values, dtype=np.float32).reshape(NPOS, H, E)
    in_maps = make_in_maps(q, k, v)
    nc = get_nc(REPS)
    res = run_bass_kernel_spmd(nc, in_maps, list(range(NCORES)))
    return unpack_out(res.results)


if __name__ == "__main__":
    rng = np.random.default_rng(0)
    qq = rng.standard_normal((B, L, H, E), dtype=np.float32)
    out = kernel(queries=qq, keys=qq, values=qq, attn_mask=0)
    print(out.shape, out.dtype)
